# revision 77
# baseline (speedup 1.0000x reference)
"""Trainium2 Bass kernel for gated-attention pooling (nn_AttentionGated).

Computation (reference):
    h = relu(x[0] @ W_feat.T + b_feat)        # [N, 768]
    a = relu(h @ W_a.T)                        # [N, 128]
    b = sigmoid(h @ W_b.T)                     # [N, 128]
    logits = (a*b) @ W_c.T                     # [N] -> softmax over N
    out = softmax(logits) @ h                  # [1, 768]

Strategy: shard N=50000 rows over 8 cores (6250 each). Each core streams its
shard in 49 tiles of 128 rows, computing h (bf16), the gated attention logit
per row, w = exp(logit), and accumulating P = sum_n w_n * h_n and Z = sum_n w_n
in persistent PSUM banks via rank-1 matmuls. No max-subtraction is needed:
logits are O(1) for normalized weights, so exp() cannot overflow. The host
merges the 8 partial (P, Z) pairs: out = sum(P_i) / sum(Z_i). This avoids any
on-device collective.

Layouts: the host pre-transposes x into per-tile [d, n]-chunk layout so the
h-GEMM can use x chunks as the stationary operand (out = x_tile @ W_feat.T in
natural [n, e] layout). h is written in bf16; hT (the stationary operand of
the a/b GEMMs, which contract over e) is produced by 6 PE transpose-mode
matmuls per tile into a single shared PSUM bank (bf16: 6 x 256B per partition
fits one 2KB bank), evacuated by one ScalarE copy. All matmuls are bf16
(1 cycle/row on TRN2; fp32 matmul is 4 cycles/row) with fp32 PSUM accumulate;
the bf16 rounding noise averages out across the 50k-row softmax pooling
(measured output rel err ~1.3e-4).
"""

import sys
import types

import numpy as np
import ml_dtypes

import concourse.bass as bass
import concourse.bacc as bacc
import concourse.mybir as mybir
from concourse import tile
from concourse.bass_utils import run_bass_kernel_spmd

BF16 = ml_dtypes.bfloat16

N_CORES = 8
N = 50000
DIM = 768
D_ATT = 128
NS = N // N_CORES            # 6250 rows per core
T = 49                       # tiles of 128 rows (6272 padded)
NP = T * 128                 # 6272
LAST_VALID = NS - (T - 1) * 128  # 106 valid rows in the last tile
NC_CHUNKS = DIM // 128       # 6

_cached_nc = None
last_results = None  # BassKernelResults of the most recent run (for profiling)


def _build_nc():
    AF = mybir.ActivationFunctionType
    dt = mybir.dt

    nc = bacc.Bacc("TRN2", target_bir_lowering=False, debug=False)

    xt_d = nc.dram_tensor("xt", [T, 128, DIM], dt.bfloat16, kind="ExternalInput").ap()
    wt_d = nc.dram_tensor("wt", [128, NC_CHUNKS, DIM], dt.bfloat16, kind="ExternalInput").ap()
    wab_d = nc.dram_tensor("wab", [128, NC_CHUNKS, 2 * D_ATT], dt.bfloat16, kind="ExternalInput").ap()
    wcbc_d = nc.dram_tensor("wcbc", [128, D_ATT], dt.bfloat16, kind="ExternalInput").ap()
    bfz_d = nc.dram_tensor("bfz", [128, DIM], dt.bfloat16, kind="ExternalInput").ap()
    mask_d = nc.dram_tensor("mask", [128, 1], dt.bfloat16, kind="ExternalInput").ap()
    ident_d = nc.dram_tensor("ident", [128, 128], dt.bfloat16, kind="ExternalInput").ap()
    out_d = nc.dram_tensor("out", [1, DIM + 1], dt.float32, kind="ExternalOutput").ap()

    with tile.TileContext(nc) as tc:
        with (
            tc.tile_pool(name="const", bufs=1) as constp,
            tc.tile_pool(name="xtp", bufs=8) as xtp,
            tc.tile_pool(name="hp", bufs=6) as hp,
            tc.tile_pool(name="Hp", bufs=4) as Hp,
            tc.tile_pool(name="smp", bufs=6) as smp,
            tc.tile_pool(name="psh", bufs=2, space="PSUM") as pshp,
            tc.tile_pool(name="psab", bufs=1, space="PSUM") as psabp,
            tc.tile_pool(name="psT", bufs=1, space="PSUM") as psTp,
            tc.tile_pool(name="psacc", bufs=1, space="PSUM") as paccp,
        ):
            # --- constants (loaded once) ---
            wt_sb = constp.tile([128, NC_CHUNKS, DIM], dt.bfloat16)
            nc.sync.dma_start(wt_sb[:, 0:3, :], wt_d[:, 0:3, :])
            nc.sync.dma_start(wt_sb[:, 3:6, :], wt_d[:, 3:6, :])
            wab_sb = constp.tile([128, NC_CHUNKS, 2 * D_ATT], dt.bfloat16)
            nc.sync.dma_start(wab_sb[:], wab_d[:])
            wcbc_sb = constp.tile([128, D_ATT], dt.bfloat16)
            nc.sync.dma_start(wcbc_sb[:], wcbc_d[:])
            # bias as a K=128 rank-1: row 0 of bfz is b_feat, rest zeros.
            # A K=1 matmul (row_grp q0) issues ~100ns slower per MM and
            # breaks the MM pipeline; the K=128 form streams at full rate.
            bfz_sb = constp.tile([128, DIM], dt.bfloat16)
            nc.sync.dma_start(bfz_sb[:], bfz_d[:])
            onesm_sb = constp.tile([128, 128], dt.bfloat16)
            nc.vector.memset(onesm_sb[:], 1.0)
            mask_sb = constp.tile([128, 1], dt.bfloat16)
            nc.sync.dma_start(mask_sb[:], mask_d[:])
            ident_sb = constp.tile([128, 128], dt.bfloat16)
            nc.sync.dma_start(ident_sb[:], ident_d[:])

            # persistent PSUM accumulators: P[0:384] | [P[384:768], Z]
            ppza = paccp.tile([1, 384], dt.float32, tag="ppza")
            ppzb = paccp.tile([1, 385], dt.float32, tag="ppzb")

            for t in range(T):
                first = t == 0
                last = t == T - 1

                xt = xtp.tile([128, DIM], dt.bfloat16, tag="xt")
                nc.sync.dma_start(xt[:], xt_d[t])

                # h = relu(x @ W_feat.T + b_feat), natural layout [n, e].
                # One [128, 2, 512] psum tile: each 512-f32 half is exactly
                # one bank (matmuls stay within-bank), and a single strided
                # ACT relu evacuates both halves in one op.
                ph = pshp.tile([128, 2, 512], dt.float32, tag="ph")
                for c in range(NC_CHUNKS):
                    lhs = xt[:, bass.ts(c, 128)]
                    nc.tensor.matmul(ph[:, 0, 0:384], lhs, wt_sb[:, c, 0:384],
                                     start=(c == 0), stop=False, skip_group_check=True)
                    nc.tensor.matmul(ph[:, 1, 0:384], lhs, wt_sb[:, c, 384:768],
                                     start=(c == 0), stop=False, skip_group_check=True)
                # bias via rank-1 matmul (only row 0 of bfz is nonzero)
                nc.tensor.matmul(ph[:, 0, 0:384], onesm_sb[:], bfz_sb[:, 0:384],
                                 start=False, stop=True, skip_group_check=True)
                nc.tensor.matmul(ph[:, 1, 0:384], onesm_sb[:], bfz_sb[:, 384:768],
                                 start=False, stop=True, skip_group_check=True)

                h = hp.tile([128, DIM + 1], dt.bfloat16, tag="h")
                nc.scalar.activation(
                    h[:, 0:768].rearrange("p (a b) -> p a b", a=2),
                    ph[:, :, 0:384], AF.Relu)
                # column 768 = softmax-denominator ones column
                if last:
                    nc.vector.tensor_copy(h[:, 768:769], mask_sb[:])
                else:
                    nc.vector.memset(h[:, 768:769], 1.0)

                # hT chunks via PE transpose-mode into one PSUM bank (bf16:
                # 6 x 256B fits a single 2KB bank), then one DVE evacuation.
                psT = psTp.tile([128, NC_CHUNKS, 128], dt.bfloat16, tag="psT")
                for c in range(NC_CHUNKS):
                    nc.tensor.transpose(psT[:, c, :], h[:, bass.ts(c, 128)],
                                        ident_sb[:])
                # evacuate on ACT: on DVE this copy queues behind the ~1us
                # reciprocal of the previous tile, delaying the a/b matmuls
                Ht = Hp.tile([128, DIM], dt.bfloat16, tag="Ht")
                nc.scalar.copy(Ht[:], psT[:])

                # aT = relu(Wa @ h^T), bT = sigmoid(Wb @ h^T)   [k, n] layout
                # [a | b] natural [n, 2k]: lhsT = Ht chunks (6 LDWs), moving
                # operand = packed [WaT | WbT] chunks (N=256 per MM).
                pab = psabp.tile([128, 2 * D_ATT], dt.float32, tag="pab")
                for c in range(NC_CHUNKS):
                    nc.tensor.matmul(pab[:], Ht[:, bass.ts(c, 128)],
                                     wab_sb[:, c, :],
                                     start=(c == 0), stop=(c == NC_CHUNKS - 1),
                                     skip_group_check=True)

                # sigmoid(u) = 1/(1+exp(-u)) via ACT Exp + DVE reciprocal:
                # keeps every ACT op in the exp_and_friends table set (no
                # ~1.3us ACT_TABLE_LOAD thrash between Sigmoid and Exp).
                a_sb = smp.tile([128, 128], dt.bfloat16, tag="a")
                eb_sb = smp.tile([128, 128], dt.bfloat16, tag="eb")
                den_sb = smp.tile([128, 128], dt.bfloat16, tag="den")
                b_sb = smp.tile([128, 128], dt.bfloat16, tag="b")
                g_sb = smp.tile([128, 128], dt.bfloat16, tag="g")
                gw_sb = smp.tile([128, 128], dt.bfloat16, tag="gw")
                lt_sb = smp.tile([128, 1], dt.float32, tag="lt")
                nc.scalar.activation(a_sb[:], pab[:, 0:128], AF.Relu)
                nc.scalar.activation(eb_sb[:], pab[:, 128:256], AF.Exp, scale=-1.0)
                nc.vector.tensor_scalar_add(den_sb[:], eb_sb[:], 1.0)
                with nc.allow_low_precision("sigmoid denominator, bf16 ok"):
                    nc.vector.reciprocal(b_sb[:], den_sb[:])
                nc.vector.tensor_mul(g_sb[:], a_sb[:], b_sb[:])
                # logits l[n] = sum_k g[n, k] * Wc[k] on DVE (mul then reduce)
                nc.vector.tensor_mul(gw_sb[:], g_sb[:], wcbc_sb[:])
                nc.vector.tensor_reduce(lt_sb[:], gw_sb[:],
                                        mybir.AxisListType.X,
                                        mybir.AluOpType.add)

                w_sb = smp.tile([128, 1], dt.bfloat16, tag="w")
                nc.scalar.activation(w_sb[:], lt_sb[:], AF.Exp)
                if last:
                    wm_sb = smp.tile([128, 1], dt.bfloat16, tag="wm")
                    nc.vector.tensor_mul(wm_sb[:], w_sb[:], mask_sb[:])
                    w_use = wm_sb
                else:
                    w_use = w_sb

                # P += w^T @ h ; Z += w^T @ ones  (persistent accumulation)
                nc.tensor.matmul(ppza[:], w_use[:], h[:, 0:384],
                                 start=first, stop=last, skip_group_check=True)
                nc.tensor.matmul(ppzb[:], w_use[:], h[:, 384:769],
                                 start=first, stop=last, skip_group_check=True)

            out_sb = constp.tile([1, DIM + 1], dt.float32)
            nc.vector.tensor_copy(out_sb[:, 0:384], ppza[:])
            nc.vector.tensor_copy(out_sb[:, 384:769], ppzb[:])
            nc.sync.dma_start(out_d[:], out_sb[:])

    nc.compile()
    return nc


def get_nc():
    global _cached_nc
    if _cached_nc is None:
        _cached_nc = _build_nc()
    return _cached_nc


def make_inputs(x, W_feat, b_feat, W_a, W_b, W_c):
    """Host-side preprocessing: shard + retile x, prepack weights."""
    x = np.asarray(x, dtype=np.float32)
    xs = x.reshape(N, DIM)
    xp = np.zeros((N_CORES, NP, DIM), dtype=np.float32)
    xp[:, :NS, :] = xs.reshape(N_CORES, NS, DIM)
    # per tile: block [128 n, 768 d] -> [p, c, n] with d = c*128 + p
    blocks = xp.reshape(N_CORES, T, 128, NC_CHUNKS, 128)     # [core, t, n, c, p]
    xt_host = np.ascontiguousarray(blocks.transpose(0, 1, 4, 3, 2)) \
        .reshape(N_CORES, T, 128, DIM).astype(BF16)

    WT = np.asarray(W_feat, np.float32).T                    # [d, e]
    wt_host = np.ascontiguousarray(
        WT.reshape(NC_CHUNKS, 128, DIM).transpose(1, 0, 2)).astype(BF16)

    wab = np.concatenate([np.asarray(W_a, np.float32).T,
                          np.asarray(W_b, np.float32).T], axis=1)  # [e, 256]
    wab_host = np.ascontiguousarray(
        wab.reshape(NC_CHUNKS, 128, 2 * D_ATT).transpose(1, 0, 2)).astype(BF16)

    wcbc_host = np.ascontiguousarray(np.tile(
        np.asarray(W_c, np.float32).reshape(1, D_ATT), (128, 1))).astype(BF16)
    bfz_host = np.zeros((128, DIM), dtype=BF16)
    bfz_host[0] = np.asarray(b_feat, np.float32).astype(BF16)
    mask_host = np.zeros((128, 1), dtype=BF16)
    mask_host[:LAST_VALID] = 1
    ident_host = np.eye(128, dtype=BF16)

    common = dict(wt=wt_host, wab=wab_host, wcbc=wcbc_host, bfz=bfz_host,
                  mask=mask_host, ident=ident_host)
    return [dict(xt=np.ascontiguousarray(xt_host[i]), **common)
            for i in range(N_CORES)]


def _ensure_axon_profile_hook():
    """If someone runs kernel() with BASS_TRACE=1 under axon, the spmd runner
    imports antenv.axon_hooks, which this image lacks; shim it from
    trn_agent_boot so tracing degrades gracefully instead of crashing."""
    try:
        import antenv.axon_hooks  # noqa: F401
        return
    except ImportError:
        pass
    try:
        from trn_agent_boot import trn_boot

        hook = trn_boot._ntff_profile_via_ctypes("/opt/axon/libaxon_pjrt.so")
        mod = types.ModuleType("antenv.axon_hooks")
        mod.get_axon_ntff_profile_hook = lambda: hook
        mod.set_axon_ntff_profile_hook = lambda h: None
        sys.modules["antenv.axon_hooks"] = mod
    except Exception:
        pass


def kernel(x, W_feat, b_feat, W_a, W_b, W_c):
    global last_results
    _ensure_axon_profile_hook()
    nc = get_nc()
    in_maps = make_inputs(x, W_feat, b_feat, W_a, W_b, W_c)
    res = run_bass_kernel_spmd(nc, in_maps, core_ids=list(range(N_CORES)))
    last_results = res
    P = np.zeros(DIM, dtype=np.float64)
    Z = 0.0
    for r in res.results:
        o = np.asarray(r["out"], dtype=np.float64).reshape(DIM + 1)
        P += o[:DIM]
        Z += o[DIM]
    return (P / Z).astype(np.float32).reshape(1, DIM)


# revision 78
# speedup vs baseline: 1.0803x; 1.0803x over previous
"""Trainium2 Bass kernel for gated-attention pooling (nn_AttentionGated).

Computation (reference):
    h = relu(x[0] @ W_feat.T + b_feat)        # [N, 768]
    a = relu(h @ W_a.T)                        # [N, 128]
    b = sigmoid(h @ W_b.T)                     # [N, 128]
    logits = (a*b) @ W_c.T                     # [N] -> softmax over N
    out = softmax(logits) @ h                  # [1, 768]

Strategy: shard N=50000 rows over 8 cores (6250 each). Each core streams its
shard in 49 tiles of 128 rows, computing h (bf16), the gated attention logit
per row, w = exp(logit), and accumulating P = sum_n w_n * h_n and Z = sum_n w_n
in persistent PSUM banks via rank-1 matmuls. No max-subtraction is needed:
logits are O(1) for normalized weights, so exp() cannot overflow. The host
merges the 8 partial (P, Z) pairs: out = sum(P_i) / sum(Z_i). This avoids any
on-device collective.

Layouts: the host pre-transposes x into per-tile [d, n]-chunk layout so the
h-GEMM can use x chunks as the stationary operand (out = x_tile @ W_feat.T in
natural [n, e] layout). h is written in bf16; hT (the stationary operand of
the a/b GEMMs, which contract over e) is produced by 6 PE transpose-mode
matmuls per tile into a single shared PSUM bank (bf16: 6 x 256B per partition
fits one 2KB bank), evacuated by one ScalarE copy. All matmuls are bf16
(1 cycle/row on TRN2; fp32 matmul is 4 cycles/row) with fp32 PSUM accumulate;
the bf16 rounding noise averages out across the 50k-row softmax pooling
(measured output rel err ~1.3e-4).
"""

import sys
import types

import numpy as np
import ml_dtypes

import concourse.bass as bass
import concourse.bacc as bacc
import concourse.mybir as mybir
from concourse import tile
from concourse.bass_utils import run_bass_kernel_spmd

BF16 = ml_dtypes.bfloat16

N_CORES = 8
N = 50000
DIM = 768
D_ATT = 128
NS = N // N_CORES            # 6250 rows per core
T = 49                       # tiles of 128 rows (6272 padded)
NP = T * 128                 # 6272
LAST_VALID = NS - (T - 1) * 128  # 106 valid rows in the last tile
NC_CHUNKS = DIM // 128       # 6

_cached_nc = None
last_results = None  # BassKernelResults of the most recent run (for profiling)


def _build_nc():
    AF = mybir.ActivationFunctionType
    dt = mybir.dt

    nc = bacc.Bacc("TRN2", target_bir_lowering=False, debug=False)

    xt_d = nc.dram_tensor("xt", [T, 128, DIM], dt.bfloat16, kind="ExternalInput").ap()
    wt_d = nc.dram_tensor("wt", [128, NC_CHUNKS, DIM], dt.bfloat16, kind="ExternalInput").ap()
    wab_d = nc.dram_tensor("wab", [128, NC_CHUNKS, 2 * D_ATT], dt.bfloat16, kind="ExternalInput").ap()
    wcbc_d = nc.dram_tensor("wcbc", [128, D_ATT], dt.bfloat16, kind="ExternalInput").ap()
    bfz_d = nc.dram_tensor("bfz", [128, DIM], dt.bfloat16, kind="ExternalInput").ap()
    mask_d = nc.dram_tensor("mask", [128, 1], dt.bfloat16, kind="ExternalInput").ap()
    ident_d = nc.dram_tensor("ident", [128, 128], dt.bfloat16, kind="ExternalInput").ap()
    out_d = nc.dram_tensor("out", [1, DIM + 1], dt.float32, kind="ExternalOutput").ap()

    with tile.TileContext(nc) as tc:
        with (
            tc.tile_pool(name="const", bufs=1) as constp,
            tc.tile_pool(name="xtp", bufs=8) as xtp,
            tc.tile_pool(name="hp", bufs=6) as hp,
            tc.tile_pool(name="Hp", bufs=4) as Hp,
            tc.tile_pool(name="smp", bufs=6) as smp,
            tc.tile_pool(name="psh", bufs=2, space="PSUM") as pshp,
            tc.tile_pool(name="psab", bufs=1, space="PSUM") as psabp,
            tc.tile_pool(name="psT", bufs=1, space="PSUM") as psTp,
            tc.tile_pool(name="psacc", bufs=1, space="PSUM") as paccp,
        ):
            # --- constants (loaded once) ---
            wt_sb = constp.tile([128, NC_CHUNKS, DIM], dt.bfloat16)
            nc.sync.dma_start(wt_sb[:, 0:3, :], wt_d[:, 0:3, :])
            nc.sync.dma_start(wt_sb[:, 3:6, :], wt_d[:, 3:6, :])
            wab_sb = constp.tile([128, NC_CHUNKS, 2 * D_ATT], dt.bfloat16)
            nc.sync.dma_start(wab_sb[:], wab_d[:])
            wcbc_sb = constp.tile([128, D_ATT], dt.bfloat16)
            nc.sync.dma_start(wcbc_sb[:], wcbc_d[:])
            # bias as a K=128 rank-1: row 0 of bfz is b_feat, rest zeros.
            # A K=1 matmul (row_grp q0) issues ~100ns slower per MM and
            # breaks the MM pipeline; the K=128 form streams at full rate.
            bfz_sb = constp.tile([128, DIM], dt.bfloat16)
            nc.sync.dma_start(bfz_sb[:], bfz_d[:])
            onesm_sb = constp.tile([128, 128], dt.bfloat16)
            nc.vector.memset(onesm_sb[:], 1.0)
            mask_sb = constp.tile([128, 1], dt.bfloat16)
            nc.sync.dma_start(mask_sb[:], mask_d[:])
            ident_sb = constp.tile([128, 128], dt.bfloat16)
            nc.sync.dma_start(ident_sb[:], ident_d[:])

            # persistent PSUM accumulators: P[0:384] | [P[384:768], Z]
            ppza = paccp.tile([1, 384], dt.float32, tag="ppza")
            ppzb = paccp.tile([1, 385], dt.float32, tag="ppzb")

            for t in range(T):
                first = t == 0
                last = t == T - 1

                xt = xtp.tile([128, DIM], dt.bfloat16, tag="xt")
                nc.sync.dma_start(xt[:], xt_d[t])

                # h = relu(x @ W_feat.T + b_feat), natural layout [n, e]
                ph0 = pshp.tile([128, 384], dt.float32, tag="ph0")
                ph1 = pshp.tile([128, 384], dt.float32, tag="ph1")
                for c in range(NC_CHUNKS):
                    lhs = xt[:, bass.ts(c, 128)]
                    nc.tensor.matmul(ph0[:], lhs, wt_sb[:, c, 0:384],
                                     start=(c == 0), stop=False, skip_group_check=True)
                    nc.tensor.matmul(ph1[:], lhs, wt_sb[:, c, 384:768],
                                     start=(c == 0), stop=False, skip_group_check=True)
                # bias via rank-1 matmul (only row 0 of bfz is nonzero)
                nc.tensor.matmul(ph0[:], onesm_sb[:], bfz_sb[:, 0:384],
                                 start=False, stop=True, skip_group_check=True)
                nc.tensor.matmul(ph1[:], onesm_sb[:], bfz_sb[:, 384:768],
                                 start=False, stop=True, skip_group_check=True)

                h = hp.tile([128, DIM + 1], dt.bfloat16, tag="h")
                nc.scalar.activation(h[:, 0:384], ph0[:], AF.Relu)
                nc.scalar.activation(h[:, 384:768], ph1[:], AF.Relu)
                # column 768 = softmax-denominator ones column
                if last:
                    nc.vector.tensor_copy(h[:, 768:769], mask_sb[:])
                else:
                    nc.vector.memset(h[:, 768:769], 1.0)

                # hT chunks via PE transpose-mode into one PSUM bank (bf16:
                # 6 x 256B fits a single 2KB bank), then one DVE evacuation.
                psT = psTp.tile([128, NC_CHUNKS, 128], dt.bfloat16, tag="psT")
                for c in range(NC_CHUNKS):
                    nc.tensor.transpose(psT[:, c, :], h[:, bass.ts(c, 128)],
                                        ident_sb[:])
                # evacuate on ACT: on DVE this copy queues behind the ~1us
                # reciprocal of the previous tile, delaying the a/b matmuls
                Ht = Hp.tile([128, DIM], dt.bfloat16, tag="Ht")
                nc.scalar.copy(Ht[:], psT[:])

                # aT = relu(Wa @ h^T), bT = sigmoid(Wb @ h^T)   [k, n] layout
                # [a | b] natural [n, 2k]: lhsT = Ht chunks (6 LDWs), moving
                # operand = packed [WaT | WbT] chunks (N=256 per MM).
                pab = psabp.tile([128, 2 * D_ATT], dt.float32, tag="pab")
                for c in range(NC_CHUNKS):
                    nc.tensor.matmul(pab[:], Ht[:, bass.ts(c, 128)],
                                     wab_sb[:, c, :],
                                     start=(c == 0), stop=(c == NC_CHUNKS - 1),
                                     skip_group_check=True)

                # sigmoid(u) = 1/(1+exp(-u)) via ACT Exp + DVE reciprocal:
                # keeps every ACT op in the exp_and_friends table set (no
                # ~1.3us ACT_TABLE_LOAD thrash between Sigmoid and Exp).
                a_sb = smp.tile([128, 128], dt.bfloat16, tag="a")
                eb_sb = smp.tile([128, 128], dt.bfloat16, tag="eb")
                den_sb = smp.tile([128, 128], dt.bfloat16, tag="den")
                b_sb = smp.tile([128, 128], dt.bfloat16, tag="b")
                g_sb = smp.tile([128, 128], dt.bfloat16, tag="g")
                gw_sb = smp.tile([128, 128], dt.bfloat16, tag="gw")
                lt_sb = smp.tile([128, 1], dt.float32, tag="lt")
                nc.scalar.activation(a_sb[:], pab[:, 0:128], AF.Relu)
                nc.scalar.activation(eb_sb[:], pab[:, 128:256], AF.Exp, scale=-1.0)
                nc.vector.tensor_scalar_add(den_sb[:], eb_sb[:], 1.0)
                with nc.allow_low_precision("sigmoid denominator, bf16 ok"):
                    nc.vector.reciprocal(b_sb[:], den_sb[:])
                nc.vector.tensor_mul(g_sb[:], a_sb[:], b_sb[:])
                # logits l[n] = sum_k g[n, k] * Wc[k] on DVE (mul then reduce)
                nc.vector.tensor_mul(gw_sb[:], g_sb[:], wcbc_sb[:])
                nc.vector.tensor_reduce(lt_sb[:], gw_sb[:],
                                        mybir.AxisListType.X,
                                        mybir.AluOpType.add)

                w_sb = smp.tile([128, 1], dt.bfloat16, tag="w")
                nc.scalar.activation(w_sb[:], lt_sb[:], AF.Exp)
                if last:
                    wm_sb = smp.tile([128, 1], dt.bfloat16, tag="wm")
                    nc.vector.tensor_mul(wm_sb[:], w_sb[:], mask_sb[:])
                    w_use = wm_sb
                else:
                    w_use = w_sb

                # P += w^T @ h ; Z += w^T @ ones  (persistent accumulation)
                nc.tensor.matmul(ppza[:], w_use[:], h[:, 0:384],
                                 start=first, stop=last, skip_group_check=True)
                nc.tensor.matmul(ppzb[:], w_use[:], h[:, 384:769],
                                 start=first, stop=last, skip_group_check=True)

            out_sb = constp.tile([1, DIM + 1], dt.float32)
            nc.vector.tensor_copy(out_sb[:, 0:384], ppza[:])
            nc.vector.tensor_copy(out_sb[:, 384:769], ppzb[:])
            nc.sync.dma_start(out_d[:], out_sb[:])

    nc.compile()
    return nc


def get_nc():
    global _cached_nc
    if _cached_nc is None:
        _cached_nc = _build_nc()
    return _cached_nc


def make_inputs(x, W_feat, b_feat, W_a, W_b, W_c):
    """Host-side preprocessing: shard + retile x, prepack weights."""
    x = np.asarray(x, dtype=np.float32)
    xs = x.reshape(N, DIM)
    xp = np.zeros((N_CORES, NP, DIM), dtype=np.float32)
    xp[:, :NS, :] = xs.reshape(N_CORES, NS, DIM)
    # per tile: block [128 n, 768 d] -> [p, c, n] with d = c*128 + p
    blocks = xp.reshape(N_CORES, T, 128, NC_CHUNKS, 128)     # [core, t, n, c, p]
    xt_host = np.ascontiguousarray(blocks.transpose(0, 1, 4, 3, 2)) \
        .reshape(N_CORES, T, 128, DIM).astype(BF16)

    WT = np.asarray(W_feat, np.float32).T                    # [d, e]
    wt_host = np.ascontiguousarray(
        WT.reshape(NC_CHUNKS, 128, DIM).transpose(1, 0, 2)).astype(BF16)

    wab = np.concatenate([np.asarray(W_a, np.float32).T,
                          np.asarray(W_b, np.float32).T], axis=1)  # [e, 256]
    wab_host = np.ascontiguousarray(
        wab.reshape(NC_CHUNKS, 128, 2 * D_ATT).transpose(1, 0, 2)).astype(BF16)

    wcbc_host = np.ascontiguousarray(np.tile(
        np.asarray(W_c, np.float32).reshape(1, D_ATT), (128, 1))).astype(BF16)
    bfz_host = np.zeros((128, DIM), dtype=BF16)
    bfz_host[0] = np.asarray(b_feat, np.float32).astype(BF16)
    mask_host = np.zeros((128, 1), dtype=BF16)
    mask_host[:LAST_VALID] = 1
    ident_host = np.eye(128, dtype=BF16)

    common = dict(wt=wt_host, wab=wab_host, wcbc=wcbc_host, bfz=bfz_host,
                  mask=mask_host, ident=ident_host)
    return [dict(xt=np.ascontiguousarray(xt_host[i]), **common)
            for i in range(N_CORES)]


def _ensure_axon_profile_hook():
    """If someone runs kernel() with BASS_TRACE=1 under axon, the spmd runner
    imports antenv.axon_hooks, which this image lacks; shim it from
    trn_agent_boot so tracing degrades gracefully instead of crashing."""
    try:
        import antenv.axon_hooks  # noqa: F401
        return
    except ImportError:
        pass
    try:
        from trn_agent_boot import trn_boot

        hook = trn_boot._ntff_profile_via_ctypes("/opt/axon/libaxon_pjrt.so")
        mod = types.ModuleType("antenv.axon_hooks")
        mod.get_axon_ntff_profile_hook = lambda: hook
        mod.set_axon_ntff_profile_hook = lambda h: None
        sys.modules["antenv.axon_hooks"] = mod
    except Exception:
        pass


def kernel(x, W_feat, b_feat, W_a, W_b, W_c):
    global last_results
    _ensure_axon_profile_hook()
    nc = get_nc()
    in_maps = make_inputs(x, W_feat, b_feat, W_a, W_b, W_c)
    res = run_bass_kernel_spmd(nc, in_maps, core_ids=list(range(N_CORES)))
    last_results = res
    P = np.zeros(DIM, dtype=np.float64)
    Z = 0.0
    for r in res.results:
        o = np.asarray(r["out"], dtype=np.float64).reshape(DIM + 1)
        P += o[:DIM]
        Z += o[DIM]
    return (P / Z).astype(np.float32).reshape(1, DIM)


# revision 81
# speedup vs baseline: 1.2481x; 1.1553x over previous
"""Trainium2 Bass kernel for gated-attention pooling (nn_AttentionGated).

Computation (reference):
    h = relu(x[0] @ W_feat.T + b_feat)        # [N, 768]
    a = relu(h @ W_a.T)                        # [N, 128]
    b = sigmoid(h @ W_b.T)                     # [N, 128]
    logits = (a*b) @ W_c.T                     # [N] -> softmax over N
    out = softmax(logits) @ h                  # [1, 768]

Strategy: shard N=50000 rows over 8 cores (6250 each). Each core streams its
shard in 49 tiles of 128 rows, computing h (bf16), the gated attention logit
per row, w = exp(logit), and accumulating P = sum_n w_n * h_n and Z = sum_n w_n
in persistent PSUM banks via rank-1 matmuls. No max-subtraction is needed:
logits are O(1) for normalized weights, so exp() cannot overflow. The host
merges the 8 partial (P, Z) pairs: out = sum(P_i) / sum(Z_i). This avoids any
on-device collective.

Layouts: the host pre-transposes x into per-tile [d, n]-chunk layout so the
h-GEMM can use x chunks as the stationary operand (out = x_tile @ W_feat.T in
natural [n, e] layout). h is written in bf16; hT (the stationary operand of
the a/b GEMMs, which contract over e) is produced by 6 PE transpose-mode
matmuls per tile into a single shared PSUM bank (bf16: 6 x 256B per partition
fits one 2KB bank), evacuated by one ScalarE copy. All matmuls are bf16
(1 cycle/row on TRN2; fp32 matmul is 4 cycles/row) with fp32 PSUM accumulate;
the bf16 rounding noise averages out across the 50k-row softmax pooling
(measured output rel err ~1.3e-4).
"""

import sys
import types

import numpy as np
import ml_dtypes

import concourse.bass as bass
import concourse.bacc as bacc
import concourse.mybir as mybir
from concourse import tile
from concourse.bass_utils import run_bass_kernel_spmd

BF16 = ml_dtypes.bfloat16

N_CORES = 8
N = 50000
DIM = 768
D_ATT = 128
NS = N // N_CORES            # 6250 rows per core
T = 49                       # tiles of 128 rows (6272 padded)
NP = T * 128                 # 6272
LAST_VALID = NS - (T - 1) * 128  # 106 valid rows in the last tile
NC_CHUNKS = DIM // 128       # 6

_cached_nc = None
last_results = None  # BassKernelResults of the most recent run (for profiling)


def _build_nc():
    AF = mybir.ActivationFunctionType
    dt = mybir.dt

    nc = bacc.Bacc("TRN2", target_bir_lowering=False, debug=False)

    xt_d = nc.dram_tensor("xt", [T, 128, DIM], dt.bfloat16, kind="ExternalInput").ap()
    wt_d = nc.dram_tensor("wt", [128, NC_CHUNKS, DIM], dt.bfloat16, kind="ExternalInput").ap()
    wab_d = nc.dram_tensor("wab", [128, NC_CHUNKS, 2 * D_ATT], dt.bfloat16, kind="ExternalInput").ap()
    wcbc_d = nc.dram_tensor("wcbc", [128, D_ATT], dt.bfloat16, kind="ExternalInput").ap()
    bfz_d = nc.dram_tensor("bfz", [128, DIM], dt.bfloat16, kind="ExternalInput").ap()
    mask_d = nc.dram_tensor("mask", [128, 1], dt.bfloat16, kind="ExternalInput").ap()
    ident_d = nc.dram_tensor("ident", [128, 128], dt.bfloat16, kind="ExternalInput").ap()
    out_d = nc.dram_tensor("out", [1, DIM + 1], dt.float32, kind="ExternalOutput").ap()

    with tile.TileContext(nc) as tc:
        with (
            tc.tile_pool(name="const", bufs=1) as constp,
            tc.tile_pool(name="xtp", bufs=8) as xtp,
            tc.tile_pool(name="hp", bufs=10) as hp,
            tc.tile_pool(name="ltp", bufs=2) as ltp,
            tc.tile_pool(name="Hp", bufs=4) as Hp,
            tc.tile_pool(name="smp", bufs=6) as smp,
            tc.tile_pool(name="psh", bufs=2, space="PSUM") as pshp,
            tc.tile_pool(name="psab", bufs=1, space="PSUM") as psabp,
            tc.tile_pool(name="psT", bufs=1, space="PSUM") as psTp,
            tc.tile_pool(name="psacc", bufs=1, space="PSUM") as paccp,
        ):
            # --- constants (loaded once) ---
            wt_sb = constp.tile([128, NC_CHUNKS, DIM], dt.bfloat16)
            nc.sync.dma_start(wt_sb[:, 0:3, :], wt_d[:, 0:3, :])
            nc.sync.dma_start(wt_sb[:, 3:6, :], wt_d[:, 3:6, :])
            wab_sb = constp.tile([128, NC_CHUNKS, 2 * D_ATT], dt.bfloat16)
            nc.sync.dma_start(wab_sb[:], wab_d[:])
            wcbc_sb = constp.tile([128, D_ATT], dt.bfloat16)
            nc.sync.dma_start(wcbc_sb[:], wcbc_d[:])
            # bias as a K=128 rank-1: row 0 of bfz is b_feat, rest zeros.
            # A K=1 matmul (row_grp q0) issues ~100ns slower per MM and
            # breaks the MM pipeline; the K=128 form streams at full rate.
            bfz_sb = constp.tile([128, DIM], dt.bfloat16)
            nc.sync.dma_start(bfz_sb[:], bfz_d[:])
            onesm_sb = constp.tile([128, 128], dt.bfloat16)
            nc.vector.memset(onesm_sb[:], 1.0)
            mask_sb = constp.tile([128, 1], dt.bfloat16)
            nc.sync.dma_start(mask_sb[:], mask_d[:])
            ident_sb = constp.tile([128, 128], dt.bfloat16)
            nc.sync.dma_start(ident_sb[:], ident_d[:])

            # persistent PSUM accumulators: P[0:384] | [P[384:768], Z]
            ppza = paccp.tile([1, 384], dt.float32, tag="ppza")
            ppzb = paccp.tile([1, 385], dt.float32, tag="ppzb")

            # exp() is batched over BATCH tiles: logits stage into lt_stage
            # and one ACT Exp serves the whole batch, so the Sigmoid<->Exp
            # ACT-table switch (~1.3us each) happens twice per batch instead
            # of twice per tile.
            BATCH = 7
            lt_stage = None
            batch_h = []

            for t in range(T):
                first = t == 0
                last = t == T - 1
                if t % BATCH == 0:
                    lt_stage = ltp.tile([128, BATCH], dt.float32, tag="lt")
                    batch_h = []

                xt = xtp.tile([128, DIM], dt.bfloat16, tag="xt")
                nc.sync.dma_start(xt[:], xt_d[t])

                # h = relu(x @ W_feat.T + b_feat), natural layout [n, e]
                ph0 = pshp.tile([128, 384], dt.float32, tag="ph0")
                ph1 = pshp.tile([128, 384], dt.float32, tag="ph1")
                for c in range(NC_CHUNKS):
                    lhs = xt[:, bass.ts(c, 128)]
                    nc.tensor.matmul(ph0[:], lhs, wt_sb[:, c, 0:384],
                                     start=(c == 0), stop=False, skip_group_check=True)
                    nc.tensor.matmul(ph1[:], lhs, wt_sb[:, c, 384:768],
                                     start=(c == 0), stop=False, skip_group_check=True)
                # bias via rank-1 matmul (only row 0 of bfz is nonzero)
                nc.tensor.matmul(ph0[:], onesm_sb[:], bfz_sb[:, 0:384],
                                 start=False, stop=True, skip_group_check=True)
                nc.tensor.matmul(ph1[:], onesm_sb[:], bfz_sb[:, 384:768],
                                 start=False, stop=True, skip_group_check=True)

                h = hp.tile([128, DIM + 1], dt.bfloat16, tag="h")
                nc.scalar.activation(h[:, 0:384], ph0[:], AF.Relu)
                nc.scalar.activation(h[:, 384:768], ph1[:], AF.Relu)
                # column 768 = softmax-denominator ones column
                if last:
                    nc.vector.tensor_copy(h[:, 768:769], mask_sb[:])
                else:
                    nc.vector.memset(h[:, 768:769], 1.0)

                # hT chunks via PE transpose-mode into one PSUM bank (bf16:
                # 6 x 256B fits a single 2KB bank), then one DVE evacuation.
                psT = psTp.tile([128, NC_CHUNKS, 128], dt.bfloat16, tag="psT")
                for c in range(NC_CHUNKS):
                    nc.tensor.transpose(psT[:, c, :], h[:, bass.ts(c, 128)],
                                        ident_sb[:])
                # evacuate on ACT: on DVE this copy queues behind the ~1us
                # reciprocal of the previous tile, delaying the a/b matmuls
                Ht = Hp.tile([128, DIM], dt.bfloat16, tag="Ht")
                nc.scalar.copy(Ht[:], psT[:])

                # aT = relu(Wa @ h^T), bT = sigmoid(Wb @ h^T)   [k, n] layout
                # [a | b] natural [n, 2k]: lhsT = Ht chunks (6 LDWs), moving
                # operand = packed [WaT | WbT] chunks (N=256 per MM).
                pab = psabp.tile([128, 2 * D_ATT], dt.float32, tag="pab")
                for c in range(NC_CHUNKS):
                    nc.tensor.matmul(pab[:], Ht[:, bass.ts(c, 128)],
                                     wab_sb[:, c, :],
                                     start=(c == 0), stop=(c == NC_CHUNKS - 1),
                                     skip_group_check=True)

                a_sb = smp.tile([128, 128], dt.bfloat16, tag="a")
                b_sb = smp.tile([128, 128], dt.bfloat16, tag="b")
                g_sb = smp.tile([128, 128], dt.bfloat16, tag="g")
                gw_sb = smp.tile([128, 128], dt.bfloat16, tag="gw")
                nc.scalar.activation(a_sb[:], pab[:, 0:128], AF.Relu)
                nc.scalar.activation(b_sb[:], pab[:, 128:256], AF.Sigmoid)
                # logits l[n] = sum_k a*sigmoid*Wc on DVE, staged per batch
                nc.vector.tensor_mul(g_sb[:], a_sb[:], wcbc_sb[:])
                nc.vector.tensor_mul(gw_sb[:], g_sb[:], b_sb[:])
                j = t % BATCH
                nc.vector.tensor_reduce(lt_stage[:, j:j + 1], gw_sb[:],
                                        mybir.AxisListType.X,
                                        mybir.AluOpType.add)
                batch_h.append(h)

                if j == BATCH - 1:
                    w_b = smp.tile([128, BATCH], dt.bfloat16, tag="wb")
                    nc.scalar.activation(w_b[:], lt_stage[:], AF.Exp)
                    if last:
                        wm_sb = smp.tile([128, 1], dt.bfloat16, tag="wm")
                        nc.vector.tensor_mul(wm_sb[:], w_b[:, BATCH - 1:BATCH],
                                             mask_sb[:])
                    # P += w^T @ h ; Z += w^T @ ones for the whole batch
                    for bj in range(BATCH):
                        tj = t - (BATCH - 1) + bj
                        hj = batch_h[bj]
                        if last and bj == BATCH - 1:
                            w_use = wm_sb[:]
                        else:
                            w_use = w_b[:, bj:bj + 1]
                        nc.tensor.matmul(ppza[:], w_use, hj[:, 0:384],
                                         start=(tj == 0), stop=(tj == T - 1),
                                         skip_group_check=True)
                        nc.tensor.matmul(ppzb[:], w_use, hj[:, 384:769],
                                         start=(tj == 0), stop=(tj == T - 1),
                                         skip_group_check=True)

            out_sb = constp.tile([1, DIM + 1], dt.float32)
            nc.vector.tensor_copy(out_sb[:, 0:384], ppza[:])
            nc.vector.tensor_copy(out_sb[:, 384:769], ppzb[:])
            nc.sync.dma_start(out_d[:], out_sb[:])

    nc.compile()
    return nc


def get_nc():
    global _cached_nc
    if _cached_nc is None:
        _cached_nc = _build_nc()
    return _cached_nc


def make_inputs(x, W_feat, b_feat, W_a, W_b, W_c):
    """Host-side preprocessing: shard + retile x, prepack weights."""
    x = np.asarray(x, dtype=np.float32)
    xs = x.reshape(N, DIM)
    xp = np.zeros((N_CORES, NP, DIM), dtype=np.float32)
    xp[:, :NS, :] = xs.reshape(N_CORES, NS, DIM)
    # per tile: block [128 n, 768 d] -> [p, c, n] with d = c*128 + p
    blocks = xp.reshape(N_CORES, T, 128, NC_CHUNKS, 128)     # [core, t, n, c, p]
    xt_host = np.ascontiguousarray(blocks.transpose(0, 1, 4, 3, 2)) \
        .reshape(N_CORES, T, 128, DIM).astype(BF16)

    WT = np.asarray(W_feat, np.float32).T                    # [d, e]
    wt_host = np.ascontiguousarray(
        WT.reshape(NC_CHUNKS, 128, DIM).transpose(1, 0, 2)).astype(BF16)

    wab = np.concatenate([np.asarray(W_a, np.float32).T,
                          np.asarray(W_b, np.float32).T], axis=1)  # [e, 256]
    wab_host = np.ascontiguousarray(
        wab.reshape(NC_CHUNKS, 128, 2 * D_ATT).transpose(1, 0, 2)).astype(BF16)

    wcbc_host = np.ascontiguousarray(np.tile(
        np.asarray(W_c, np.float32).reshape(1, D_ATT), (128, 1))).astype(BF16)
    bfz_host = np.zeros((128, DIM), dtype=BF16)
    bfz_host[0] = np.asarray(b_feat, np.float32).astype(BF16)
    mask_host = np.zeros((128, 1), dtype=BF16)
    mask_host[:LAST_VALID] = 1
    ident_host = np.eye(128, dtype=BF16)

    common = dict(wt=wt_host, wab=wab_host, wcbc=wcbc_host, bfz=bfz_host,
                  mask=mask_host, ident=ident_host)
    return [dict(xt=np.ascontiguousarray(xt_host[i]), **common)
            for i in range(N_CORES)]


def _ensure_axon_profile_hook():
    """If someone runs kernel() with BASS_TRACE=1 under axon, the spmd runner
    imports antenv.axon_hooks, which this image lacks; shim it from
    trn_agent_boot so tracing degrades gracefully instead of crashing."""
    try:
        import antenv.axon_hooks  # noqa: F401
        return
    except ImportError:
        pass
    try:
        from trn_agent_boot import trn_boot

        hook = trn_boot._ntff_profile_via_ctypes("/opt/axon/libaxon_pjrt.so")
        mod = types.ModuleType("antenv.axon_hooks")
        mod.get_axon_ntff_profile_hook = lambda: hook
        mod.set_axon_ntff_profile_hook = lambda h: None
        sys.modules["antenv.axon_hooks"] = mod
    except Exception:
        pass


def kernel(x, W_feat, b_feat, W_a, W_b, W_c):
    global last_results
    _ensure_axon_profile_hook()
    nc = get_nc()
    in_maps = make_inputs(x, W_feat, b_feat, W_a, W_b, W_c)
    res = run_bass_kernel_spmd(nc, in_maps, core_ids=list(range(N_CORES)))
    last_results = res
    P = np.zeros(DIM, dtype=np.float64)
    Z = 0.0
    for r in res.results:
        o = np.asarray(r["out"], dtype=np.float64).reshape(DIM + 1)
        P += o[:DIM]
        Z += o[DIM]
    return (P / Z).astype(np.float32).reshape(1, DIM)
